# revision 4
# baseline (speedup 1.0000x reference)
"""Trainium2 Bass kernel for nn_Bert (VOCAB=9, D=4, S=16384) on 8 NeuronCores.

Key identity: with a tiny vocabulary (9) and tiny width (4), every row of the
reference output depends only on the token id x[s] and the *global* histogram
c_v of x:

    T = emb @ proj_w.T + proj_b                       (9,4)  per-token h1
    G = T @ T.T                                       (9,9)  symmetric score table
    attn_out(a) = sum_v c_v e^{G[a,v]} T[v] / sum_v c_v e^{G[a,v]}
    F = softmax(relu(attn_out) @ M2.T + b2)           (9,9)  final per-token table
        where M2 = prj_w @ forw_w, b2 = prj_w @ forw_b + prj_b
        (the two affine layers after the relu compose into one)
    out[s] = F[x[s]]

Device work per core (sequence row-sharded, 2048 positions/core):
  - histogram of the full x: 9 fused is_equal+accum DVE ops + 1 PE matmul
  - 9x9 table math on PE/ACT/DVE (exp fused with the c_v scale via ln(c) bias)
  - final gather as a one-hot matmul: outT[j, s] = sum_v F[v,j] * (x[s]==v)
"""

import os
from contextlib import ExitStack

import ml_dtypes
import numpy as np

import concourse.bass as bass
import concourse.tile as tile
from concourse import bacc, mybir
from concourse._compat import get_trn_type
from concourse.bass_utils import run_bass_kernel_spmd

VOCAB = 9
D = 4
S = 16384
NCORES = 8
SLICE = S // NCORES  # 2048
NCHUNK = 4           # 512-column matmul chunks of the per-core slice
CHUNK = SLICE // NCHUNK

F32 = mybir.dt.float32
BF16 = mybir.dt.bfloat16

# Packed constants layout, one [128, 33] f32 tensor:
#   col 0      : ones (rows 0..127)
#   cols 1:5   : A  = [proj_w.T; proj_b]  rows 0..4   (K=5 augmented proj)
#   cols 5:14  : B  = [emb.T; ones(9)]   rows 0..4
#   cols 14:23 : D2 = [M2.T; b2]         rows 0..4    (folded forw+classifier)
#   col 23     : iota9 (rows 0..8 = 0..8)
#   cols 24:33 : eye9 (rows 0..8)
NCONST = 33

LAST_RESULTS = None  # BassKernelResults of the most recent run (for test.py)


def build_nc():
    nc = bacc.Bacc(
        get_trn_type() or "TRN2",
        target_bir_lowering=False,
        debug=False,
        enable_asserts=False,
        num_devices=NCORES,
    )
    xall = nc.dram_tensor("xall", [128, 128], BF16, kind="ExternalInput")
    xqrep = nc.dram_tensor("xqrep", [VOCAB, SLICE], F32, kind="ExternalInput")
    consts = nc.dram_tensor("consts", [128, NCONST], F32, kind="ExternalInput")
    outT = nc.dram_tensor("outT", [VOCAB, SLICE], F32, kind="ExternalOutput")

    with tile.TileContext(nc) as tc:
        with ExitStack() as ctx:
            _build_kernel(ctx, tc, xall.ap(), xqrep.ap(), consts.ap(), outT.ap())
    nc.compile()
    return nc


def _build_kernel(ctx, tc, xall, xqrep, consts, outT):
    nc = tc.nc
    pool = ctx.enter_context(tc.tile_pool(name="sbuf", bufs=1))
    psum = ctx.enter_context(tc.tile_pool(name="psum", bufs=4, space="PSUM"))
    psum_out = ctx.enter_context(tc.tile_pool(name="psum_out", bufs=1, space="PSUM"))

    # ---- input DMAs on three different queues so they overlap ----
    const_s = pool.tile([128, NCONST], F32)
    nc.scalar.dma_start(const_s[:], consts)
    x_s = pool.tile([128, 128], BF16)
    nc.sync.dma_start(x_s[:], xall)
    xq_s = pool.tile([VOCAB, SLICE], F32)
    nc.gpsimd.dma_start(xq_s[:], xqrep)

    ones128 = const_s[0:128, 0:1]
    ones9 = const_s[0:VOCAB, 0:1]
    A_s = const_s[0:5, 1:5]
    B_s = const_s[0:5, 5:14]
    D2_s = const_s[0:5, 14:23]
    iota9 = const_s[0:VOCAB, 23:24]
    eye9 = const_s[0:VOCAB, 24:33]

    # ---- histogram of the full x: H[p, v] = sum_f (x[p,f] == v) ----
    scratch = pool.tile([128, 128], BF16)
    H = pool.tile([128, VOCAB], F32)
    for v in range(VOCAB):
        nc.vector.tensor_scalar(
            out=scratch[:],
            in0=x_s[:],
            scalar1=float(v),
            scalar2=None,
            op0=mybir.AluOpType.is_equal,
            op1=mybir.AluOpType.add,
            accum_out=H[:, v : v + 1],
        )
    c_ps = psum.tile([VOCAB, 1], F32, tag="small")
    nc.tensor.matmul(c_ps[:], H[:], ones128)  # c[v] = sum_p H[p, v]
    lnc_s = pool.tile([VOCAB, 1], F32)
    nc.scalar.activation(lnc_s[:], c_ps[:], mybir.ActivationFunctionType.Ln)

    # ---- per-token tables (all 9x9 or smaller) ----
    # T_T[d, a] (bias folded via the augmented K=5 contraction), T[a, d]
    TT_ps = psum.tile([D, VOCAB], F32, tag="small")
    nc.tensor.matmul(TT_ps[:], A_s, B_s)
    T_ps = psum.tile([VOCAB, D], F32, tag="small")
    nc.tensor.matmul(T_ps[:], B_s, A_s)
    TT_s = pool.tile([D, VOCAB], F32)
    nc.scalar.copy(TT_s[:], TT_ps[:])
    T_s = pool.tile([VOCAB, D], F32)
    nc.scalar.copy(T_s[:], T_ps[:])

    # G[a, v] = T[a] . T[v]  (symmetric)
    G_ps = psum.tile([VOCAB, VOCAB], F32, tag="small")
    nc.tensor.matmul(G_ps[:], TT_s[:], TT_s[:])

    # W[v, a] = c_v * exp(G[v, a])   (exp fused with the count scale)
    W_s = pool.tile([VOCAB, VOCAB], F32)
    nc.scalar.activation(
        W_s[:], G_ps[:], mybir.ActivationFunctionType.Exp, bias=lnc_s[:]
    )

    # Z[a] = sum_v W[v, a];  Sh[a, :] = sum_v W[v, a] T[v, :]
    Z_ps = psum.tile([VOCAB, 1], F32, tag="small")
    nc.tensor.matmul(Z_ps[:], W_s[:], ones9)
    Zr_s = pool.tile([VOCAB, 1], F32)
    nc.vector.reciprocal(Zr_s[:], Z_ps[:])
    Sh_ps = psum.tile([VOCAB, D], F32, tag="small")
    nc.tensor.matmul(Sh_ps[:], W_s[:], T_s[:])

    # h3[a, :] = relu(Sh[a, :] * Zr[a])
    h3_s = pool.tile([VOCAB, D], F32)
    nc.scalar.activation(
        h3_s[:], Sh_ps[:], mybir.ActivationFunctionType.Relu, scale=Zr_s[:]
    )

    # transpose h3 -> [d, a], augment with a ones row for the bias fold
    h3T_ps = psum.tile([D, VOCAB], F32, tag="small")
    nc.tensor.transpose(h3T_ps[:], h3_s[:], eye9)
    h3a_s = pool.tile([5, VOCAB], F32)
    nc.gpsimd.memset(h3a_s[:], 1.0)
    nc.scalar.copy(h3a_s[0:4, :], h3T_ps[:])

    # logits[a, j] = sum_d h3[a, d] M2[j, d] + b2[j]
    logit_ps = psum.tile([VOCAB, VOCAB], F32, tag="small")
    nc.tensor.matmul(logit_ps[:], h3a_s[:], D2_s)

    # F[a, j] = softmax_j(logits[a, :])  (row sum fused into the exp)
    expL_s = pool.tile([VOCAB, VOCAB], F32)
    Ssum_s = pool.tile([VOCAB, 1], F32)
    nc.scalar.activation(
        expL_s[:],
        logit_ps[:],
        mybir.ActivationFunctionType.Exp,
        accum_out=Ssum_s[:],
    )
    Sr_s = pool.tile([VOCAB, 1], F32)
    nc.vector.reciprocal(Sr_s[:], Ssum_s[:])
    F_s = pool.tile([VOCAB, VOCAB], F32)
    nc.scalar.activation(
        F_s[:], expL_s[:], mybir.ActivationFunctionType.Copy, scale=Sr_s[:]
    )

    # ---- final gather: outT[j, s] = sum_v F[v, j] * (xq[s] == v) ----
    oh_s = pool.tile([VOCAB, SLICE], F32)
    nc.vector.tensor_scalar(
        out=oh_s[:],
        in0=xq_s[:],
        scalar1=iota9,
        scalar2=None,
        op0=mybir.AluOpType.is_equal,
    )
    outT_ps = psum_out.tile([VOCAB, SLICE], F32)
    outT_s = pool.tile([VOCAB, SLICE], F32)
    for cidx in range(NCHUNK):
        sl = slice(cidx * CHUNK, (cidx + 1) * CHUNK)
        nc.tensor.matmul(outT_ps[:, sl], F_s[:], oh_s[:, sl])
        nc.scalar.copy(outT_s[:, sl], outT_ps[:, sl])
    nc.sync.dma_start(outT, outT_s[:])


def host_prep(x, emb, proj_w, proj_b, forw_w, forw_b, prj_w, prj_b):
    """Pack weights/constants and per-core sharded inputs."""
    f32 = np.float32
    x = np.asarray(x).reshape(-1).astype(np.int64)
    assert x.shape == (S,)
    emb = np.asarray(emb, f32)
    proj_w = np.asarray(proj_w, f32)
    proj_b = np.asarray(proj_b, f32)
    forw_w = np.asarray(forw_w, f32)
    forw_b = np.asarray(forw_b, f32)
    prj_w = np.asarray(prj_w, f32)
    prj_b = np.asarray(prj_b, f32)

    M2 = (prj_w @ forw_w).astype(f32)          # (9, 4)
    b2 = (prj_w @ forw_b + prj_b).astype(f32)  # (9,)

    consts = np.zeros((128, NCONST), f32)
    consts[:, 0] = 1.0
    consts[0:4, 1:5] = proj_w.T
    consts[4, 1:5] = proj_b
    consts[0:4, 5:14] = emb.T
    consts[4, 5:14] = 1.0
    consts[0:4, 14:23] = M2.T
    consts[4, 14:23] = b2
    consts[0:VOCAB, 23] = np.arange(VOCAB, dtype=f32)
    consts[0:VOCAB, 24:33] = np.eye(VOCAB, dtype=f32)

    xall = x.reshape(128, 128).astype(ml_dtypes.bfloat16)
    in_maps = []
    for i in range(NCORES):
        xq = x[i * SLICE : (i + 1) * SLICE].astype(f32)
        in_maps.append(
            {
                "xall": xall,
                "consts": consts,
                "xqrep": np.ascontiguousarray(
                    np.broadcast_to(xq[None, :], (VOCAB, SLICE))
                ),
            }
        )
    return in_maps


_NC_CACHE = None


def kernel(x, emb, proj_w, proj_b, forw_w, forw_b, prj_w, prj_b):
    global _NC_CACHE, LAST_RESULTS
    if _NC_CACHE is None:
        _NC_CACHE = build_nc()
    nc = _NC_CACHE
    in_maps = host_prep(x, emb, proj_w, proj_b, forw_w, forw_b, prj_w, prj_b)
    trace = bool(os.environ.get("BASS_TRACE"))
    res = run_bass_kernel_spmd(nc, in_maps, list(range(NCORES)), trace=trace)
    LAST_RESULTS = res
    out = np.empty((S, VOCAB), np.float32)
    for i in range(NCORES):
        out[i * SLICE : (i + 1) * SLICE, :] = res.results[i]["outT"].T
    return out


# revision 7
# speedup vs baseline: 1.2928x; 1.2928x over previous
"""Trainium2 Bass kernel for nn_Bert (VOCAB=9, D=4, S=16384) on 8 NeuronCores.

Key identity: with a tiny vocabulary (9) and tiny width (4), every row of the
reference output depends only on the token id x[s] and the *global* histogram
c_v of x:

    T = emb @ proj_w.T + proj_b                       (9,4)  per-token h1
    G = T @ T.T                                       (9,9)  symmetric score table
    attn_out(a) = sum_v c_v e^{G[a,v]} T[v] / sum_v c_v e^{G[a,v]}
    F = softmax(relu(attn_out) @ M2.T + b2)           (9,9)  final per-token table
        where M2 = prj_w @ forw_w, b2 = prj_w @ forw_b + prj_b
        (the two affine layers after the relu compose into one)
    out[s] = F[x[s]]

Device work per core (sequence row-sharded, 2048 positions/core):
  - histogram of the full x: 9 fused is_equal+accum ops (split DVE/GpSimd)
  - 9x9 table math on PE/ACT/DVE (single Exp activation table)
  - final gather as a one-hot matmul in bf16 with an exact hi/lo split of F
    (psum-accumulated), chunk-pipelined into 4 output DMAs
"""

import os
from contextlib import ExitStack

import ml_dtypes
import numpy as np

import concourse.bass as bass
import concourse.tile as tile
from concourse import bacc, mybir
from concourse._compat import get_trn_type
from concourse.bass_utils import run_bass_kernel_spmd

VOCAB = 9
D = 4
S = 16384
NCORES = 8
SLICE = S // NCORES  # 2048
NCHUNK = 4           # 512-column matmul chunks of the per-core slice
CHUNK = SLICE // NCHUNK

F32 = mybir.dt.float32
BF16 = mybir.dt.bfloat16

# Packed constants layout, one [128, 33] f32 tensor:
#   col 0      : ones (rows 0..127)
#   cols 1:5   : A  = [proj_w.T; proj_b]  rows 0..4   (K=5 augmented proj)
#   cols 5:14  : B  = [emb.T; ones(9)]   rows 0..4
#   cols 14:23 : D2 = [M2.T; b2]         rows 0..4    (folded forw+classifier)
#   col 23     : iota9 (rows 0..8 = 0..8)
#   cols 24:33 : eye9 (rows 0..8)
#   cols 33:36 : -6, -7, -8 (all rows; ACT histogram bias constants)
NCONST = 36

LAST_RESULTS = None  # BassKernelResults of the most recent run (for test.py)


def build_nc():
    nc = bacc.Bacc(
        get_trn_type() or "TRN2",
        target_bir_lowering=False,
        debug=False,
        enable_asserts=False,
        num_devices=NCORES,
    )
    xall = nc.dram_tensor("xall", [128, 128], BF16, kind="ExternalInput")
    xqrep = nc.dram_tensor("xqrep", [VOCAB, SLICE], BF16, kind="ExternalInput")
    consts = nc.dram_tensor("consts", [128, NCONST], F32, kind="ExternalInput")
    outT = nc.dram_tensor("outT", [VOCAB, SLICE], F32, kind="ExternalOutput")

    with tile.TileContext(nc) as tc:
        with ExitStack() as ctx:
            _build_kernel(ctx, tc, xall.ap(), xqrep.ap(), consts.ap(), outT.ap())
    nc.compile()
    return nc


def _build_kernel(ctx, tc, xall, xqrep, consts, outT):
    nc = tc.nc
    pool = ctx.enter_context(tc.tile_pool(name="sbuf", bufs=1))
    psum = ctx.enter_context(tc.tile_pool(name="psum", bufs=4, space="PSUM"))
    psum_out = ctx.enter_context(tc.tile_pool(name="psum_out", bufs=4, space="PSUM"))

    # ---- input DMAs on three different queues so they overlap ----
    x_s = pool.tile([128, 128], BF16)
    nc.sync.dma_start(x_s[:], xall)
    const_s = pool.tile([128, NCONST], F32)
    nc.scalar.dma_start(const_s[:], consts)
    xq_s = pool.tile([VOCAB, SLICE], BF16)
    nc.gpsimd.dma_start(xq_s[:], xqrep)

    ones128 = const_s[0:128, 0:1]
    ones9 = const_s[0:VOCAB, 0:1]
    A_s = const_s[0:5, 1:5]
    B_s = const_s[0:5, 5:14]
    D2_s = const_s[0:5, 14:23]
    iota9 = const_s[0:VOCAB, 23:24]
    eye9 = const_s[0:VOCAB, 24:33]

    # ---- histogram of the full x: H[p, v] = sum_f (x[p,f] == v) ----
    # split across DVE (is_equal) and ACT (relu(1-|x-v|), exact for integer x,
    # both Abs and Relu are table-free) so the nine passes run in parallel
    scr_v = pool.tile([128, 128], BF16)
    scr_a = pool.tile([128, 128], BF16)
    scr_b = pool.tile([128, 128], BF16)
    H = pool.tile([128, VOCAB], F32)
    for v in range(VOCAB):
        if v < 6:
            nc.vector.tensor_scalar(
                out=scr_v[:],
                in0=x_s[:],
                scalar1=float(v),
                scalar2=None,
                op0=mybir.AluOpType.is_equal,
                op1=mybir.AluOpType.add,
                accum_out=H[:, v : v + 1],
            )
        else:
            nc.scalar.activation(
                scr_a[:],
                x_s[:],
                mybir.ActivationFunctionType.Abs,
                bias=const_s[0:128, 27 + v : 28 + v],
            )
            nc.scalar.activation(
                scr_b[:],
                scr_a[:],
                mybir.ActivationFunctionType.Relu,
                bias=1.0,
                scale=-1.0,
                accum_out=H[:, v : v + 1],
            )
    c_ps = psum.tile([VOCAB, 1], F32, tag="small")
    nc.tensor.matmul(c_ps[:], H[:], ones128)  # c[v] = sum_p H[p, v]
    c_s = pool.tile([VOCAB, 1], F32)
    nc.scalar.copy(c_s[:], c_ps[:])

    # ---- one-hot for the final gather (off the critical chain) ----
    # ohT[v, s] = (xq[s] == v), bf16 (exact 0/1)
    oh_s = pool.tile([VOCAB, SLICE], BF16)
    nc.vector.tensor_scalar(
        out=oh_s[:],
        in0=xq_s[:],
        scalar1=iota9,
        scalar2=None,
        op0=mybir.AluOpType.is_equal,
    )

    # ---- per-token tables (all 9x9 or smaller) ----
    # T_T[d, a] (bias folded via the augmented K=5 contraction), T[a, d]
    TT_ps = psum.tile([D, VOCAB], F32, tag="small")
    nc.tensor.matmul(TT_ps[:], A_s, B_s)
    T_ps = psum.tile([VOCAB, D], F32, tag="small")
    nc.tensor.matmul(T_ps[:], B_s, A_s)
    TT_s = pool.tile([D, VOCAB], F32)
    nc.scalar.copy(TT_s[:], TT_ps[:])
    T_s = pool.tile([VOCAB, D], F32)
    nc.scalar.copy(T_s[:], T_ps[:])

    # G[a, v] = T[a] . T[v]  (symmetric)
    G_ps = psum.tile([VOCAB, VOCAB], F32, tag="small")
    nc.tensor.matmul(G_ps[:], TT_s[:], TT_s[:])

    # W[v, a] = c_v * exp(G[v, a])
    E_s = pool.tile([VOCAB, VOCAB], F32)
    nc.scalar.activation(E_s[:], G_ps[:], mybir.ActivationFunctionType.Exp)
    W_s = pool.tile([VOCAB, VOCAB], F32)
    nc.vector.tensor_scalar(
        out=W_s[:],
        in0=E_s[:],
        scalar1=c_s[:],
        scalar2=None,
        op0=mybir.AluOpType.mult,
    )

    # Z[a] = sum_v W[v, a];  Sh[a, :] = sum_v W[v, a] T[v, :]
    Z_ps = psum.tile([VOCAB, 1], F32, tag="small")
    nc.tensor.matmul(Z_ps[:], W_s[:], ones9)
    Zr_s = pool.tile([VOCAB, 1], F32)
    nc.vector.reciprocal(Zr_s[:], Z_ps[:])
    Sh_ps = psum.tile([VOCAB, D], F32, tag="small")
    nc.tensor.matmul(Sh_ps[:], W_s[:], T_s[:])

    # h3[a, :] = relu(Sh[a, :] * Zr[a])   (fused mult+max on DVE)
    h3_s = pool.tile([VOCAB, D], F32)
    nc.vector.tensor_scalar(
        out=h3_s[:],
        in0=Sh_ps[:],
        scalar1=Zr_s[:],
        scalar2=0.0,
        op0=mybir.AluOpType.mult,
        op1=mybir.AluOpType.max,
    )

    # transpose h3 -> [d, a], augment with a ones row for the bias fold
    h3T_ps = psum.tile([D, VOCAB], F32, tag="small")
    nc.tensor.transpose(h3T_ps[:], h3_s[:], eye9)
    h3a_s = pool.tile([5, VOCAB], F32)
    nc.gpsimd.memset(h3a_s[:], 1.0)
    nc.scalar.copy(h3a_s[0:4, :], h3T_ps[:])

    # logits[a, j] = sum_d h3[a, d] M2[j, d] + b2[j]
    logit_ps = psum.tile([VOCAB, VOCAB], F32, tag="small")
    nc.tensor.matmul(logit_ps[:], h3a_s[:], D2_s)

    # F[a, j] = softmax_j(logits[a, :])  (row sum fused into the exp)
    expL_s = pool.tile([VOCAB, VOCAB], F32)
    Ssum_s = pool.tile([VOCAB, 1], F32)
    nc.scalar.activation(
        expL_s[:],
        logit_ps[:],
        mybir.ActivationFunctionType.Exp,
        accum_out=Ssum_s[:],
    )
    Sr_s = pool.tile([VOCAB, 1], F32)
    nc.vector.reciprocal(Sr_s[:], Ssum_s[:])
    F_s = pool.tile([VOCAB, VOCAB], F32)
    nc.scalar.activation(
        F_s[:], expL_s[:], mybir.ActivationFunctionType.Copy, scale=Sr_s[:]
    )

    # exact bf16 hi/lo split of F so the gather matmul runs in bf16
    Fhi_s = pool.tile([VOCAB, VOCAB], BF16)
    nc.scalar.copy(Fhi_s[:], F_s[:])
    Flo_s = pool.tile([VOCAB, VOCAB], BF16)
    nc.vector.tensor_tensor(
        out=Flo_s[:], in0=F_s[:], in1=Fhi_s[:], op=mybir.AluOpType.subtract
    )

    # ---- final gather: outT[j, s] = sum_v F[v, j] * (xq[s] == v) ----
    outT_s = pool.tile([VOCAB, SLICE], F32)
    for cidx in range(NCHUNK):
        sl = slice(cidx * CHUNK, (cidx + 1) * CHUNK)
        o_ps = psum_out.tile([VOCAB, CHUNK], F32, tag="obank")
        nc.tensor.matmul(o_ps[:], Fhi_s[:], oh_s[:, sl], start=True, stop=False)
        nc.tensor.matmul(o_ps[:], Flo_s[:], oh_s[:, sl], start=False, stop=True)
        if cidx % 2 == 0:
            nc.scalar.copy(outT_s[:, sl], o_ps[:])
        else:
            nc.vector.tensor_copy(outT_s[:, sl], o_ps[:])
        nc.sync.dma_start(outT[:, sl], outT_s[:, sl])


def host_prep(x, emb, proj_w, proj_b, forw_w, forw_b, prj_w, prj_b):
    """Pack weights/constants and per-core sharded inputs."""
    f32 = np.float32
    x = np.asarray(x).reshape(-1).astype(np.int64)
    assert x.shape == (S,)
    emb = np.asarray(emb, f32)
    proj_w = np.asarray(proj_w, f32)
    proj_b = np.asarray(proj_b, f32)
    forw_w = np.asarray(forw_w, f32)
    forw_b = np.asarray(forw_b, f32)
    prj_w = np.asarray(prj_w, f32)
    prj_b = np.asarray(prj_b, f32)

    M2 = (prj_w @ forw_w).astype(f32)          # (9, 4)
    b2 = (prj_w @ forw_b + prj_b).astype(f32)  # (9,)

    consts = np.zeros((128, NCONST), f32)
    consts[:, 0] = 1.0
    consts[0:4, 1:5] = proj_w.T
    consts[4, 1:5] = proj_b
    consts[0:4, 5:14] = emb.T
    consts[4, 5:14] = 1.0
    consts[0:4, 14:23] = M2.T
    consts[4, 14:23] = b2
    consts[0:VOCAB, 23] = np.arange(VOCAB, dtype=f32)
    consts[0:VOCAB, 24:33] = np.eye(VOCAB, dtype=f32)
    consts[:, 33] = -6.0
    consts[:, 34] = -7.0
    consts[:, 35] = -8.0

    xall = x.reshape(128, 128).astype(ml_dtypes.bfloat16)
    in_maps = []
    for i in range(NCORES):
        xq = x[i * SLICE : (i + 1) * SLICE].astype(ml_dtypes.bfloat16)
        in_maps.append(
            {
                "xall": xall,
                "consts": consts,
                "xqrep": np.ascontiguousarray(
                    np.broadcast_to(xq[None, :], (VOCAB, SLICE))
                ),
            }
        )
    return in_maps


_NC_CACHE = None


def kernel(x, emb, proj_w, proj_b, forw_w, forw_b, prj_w, prj_b):
    global _NC_CACHE, LAST_RESULTS
    if _NC_CACHE is None:
        _NC_CACHE = build_nc()
    nc = _NC_CACHE
    in_maps = host_prep(x, emb, proj_w, proj_b, forw_w, forw_b, prj_w, prj_b)
    trace = bool(os.environ.get("BASS_TRACE"))
    res = run_bass_kernel_spmd(nc, in_maps, list(range(NCORES)), trace=trace)
    LAST_RESULTS = res
    out = np.empty((S, VOCAB), np.float32)
    for i in range(NCORES):
        out[i * SLICE : (i + 1) * SLICE, :] = res.results[i]["outT"].T
    return out
